# revision 19
# baseline (speedup 1.0000x reference)
"""Trainium2 Bass kernel for nn_MultiHeadAttention (B=2,S=2048,D=1024,H=16,HD=64).

Sharding: tensor-parallel over heads (2 heads/core x 8 cores).
Per core:
  Phase A: load pre-transposed bf16 X^T (Q/K/V inputs), project to
           Qt/Kt [128(2h*64), 4096] (transposed, bf16) and V [4096, 2x(64+ones)] (bf16).
  Phase B: per (batch, q-chunk): scores^T[keys,q] = Kt_h-tiles @ Qt_h (PE, bf16),
           exp via ACT (scale folded into Qt; no max-subtraction -- scores are
           O(1) by construction), val^T[65,q] = V_ext^T @ exp_st (ones column ->
           row 64 = softmax denom), val MMs interleaved per double-key-tile.
  Norm:    per q-chunk (scheduled inside the NEXT chunk's phase B):
           reciprocal_approx_fast on [2,512] denoms, K=2 PE broadcast to
           [128,512], in-place DVE multiply into valT, then stage + per-chunk
           AllToAll [M,F,64] (each dst core gets its 64-token strip).
  Phase D: strictly at the tail (never blocks the attention PE stream on a
           collective): out^T[o, strip] = WpT-tiles @ concatT-strips + bp.
           Batch-0 strips + batch-1 strips 0-2 first; the last chunk's a2a
           flies under ~15us of phase-D work before its strip is consumed.
Host: pure layout prep (transposes/slices/bf16 cast) + output assembly.

Per-chunk collectives (8 x 128KB + warmup) give every a2a except the last
50-150us of slack against ncfw latency variance; nothing in the attention
stream waits on a collective.
"""
import sys
import numpy as np
import ml_dtypes

sys.path.insert(0, "/opt/trn_rl_repo")
sys.path.insert(0, "/opt/trn_rl_repo/concourse")

import concourse.bass as bass
import concourse.tile as tile
from concourse import bacc, mybir
from concourse.bass_utils import run_bass_kernel_spmd

FP32 = mybir.dt.float32
BF16 = mybir.dt.bfloat16
AF = mybir.ActivationFunctionType
ALU = mybir.AluOpType

B, S, D, H, HD = 2, 2048, 1024, 16, 64
M = 8                 # cores
HC = H // M           # heads per core
F = HC * HD           # 128 per-core proj features
T = B * S             # 4096 tokens
TS = T // M           # 512 tokens per core (final proj)
SR = S // M           # 256 seq rows per core per batch
SCALE = HD ** -0.5
P = 128
NKT = D // P          # 8 contraction tiles
NST = S // P          # 16 key tiles per batch
NDK = NST // 2        # 8 double key tiles
NQC = S // 512        # 4 q-chunks per batch
NCH = B * NQC         # 8 chunks total

_CACHE = {}


def build():
    nc = bacc.Bacc("TRN2", target_bir_lowering=False, debug=False, num_devices=M)

    XT = {x: nc.dram_tensor(f"{x}T", [D, T], BF16, kind="ExternalInput") for x in "qkv"}
    W2 = {x: nc.dram_tensor(f"w2{x}", [D, F], BF16, kind="ExternalInput") for x in "qkv"}
    b2q = nc.dram_tensor("b2q", [F, 1], FP32, kind="ExternalInput")
    b2k = nc.dram_tensor("b2k", [F, 1], FP32, kind="ExternalInput")
    bvb = nc.dram_tensor("bvb", [P, HC * (HD + 1)], FP32, kind="ExternalInput")
    WpT = nc.dram_tensor("WpT", [D, D], BF16, kind="ExternalInput")
    bpT = nc.dram_tensor("bpT", [P, D // P], FP32, kind="ExternalInput")
    onehot_d = nc.dram_tensor("onehot", [HC, P], FP32, kind="ExternalInput")
    outT = nc.dram_tensor("outT", [D, TS], FP32, kind="ExternalOutput")
    # 3 consolidated a2a payloads: batch0 (4 strips), batch1 strips 0-2, strip 3
    gin = [nc.dram_tensor(f"gin{i}", [M, F, ns, 64], BF16)
           for i, ns in enumerate((4, 3, 1))]
    gout = [nc.dram_tensor(f"gout{i}", [M, F, ns, 64], BF16)
            for i, ns in enumerate((4, 3, 1))]

    with tile.TileContext(nc) as tc:
        with (
            tc.tile_pool(name="persist", bufs=1) as persist,
            tc.tile_pool(name="xtb", bufs=6) as xtbp,
            tc.tile_pool(name="est", bufs=6) as estp,
            tc.tile_pool(name="small", bufs=4) as small,
            tc.tile_pool(name="den", bufs=6) as denp,
            tc.tile_pool(name="rcp", bufs=2) as rcpp,
            tc.tile_pool(name="ps_st", bufs=3, space="PSUM") as psA,   # [128,1024] = 2 banks
            tc.tile_pool(name="ps_val", bufs=2, space="PSUM") as psV,  # [128,512] = 1 bank
        ):
            # ---------- warmup collective first (absorbs ncfw cold-start) ----
            warm_in = nc.dram_tensor("cc_warm_in", [M, 1, 64], BF16)
            warm_out = nc.dram_tensor("cc_warm_out", [M, 1, 64], BF16)
            wtile = small.tile([1, M * 64], BF16, name="wtile")
            nc.vector.memset(wtile[:], 1.0)
            nc.gpsimd.dma_start(warm_in.ap().rearrange("j p r -> p (j r)"), wtile[:])
            nc.gpsimd.collective_compute(
                "AllToAll", ALU.bypass, replica_groups=[list(range(M))],
                ins=[warm_in[:]], outs=[warm_out[:]])

            # ---------- first-consumed loads first: k0 halves + w2b_k -------
            xtb_k0 = xtbp.tile([P, NKT, 512], BF16, name="xtb")
            nc.sync.dma_start(
                xtb_k0[:, 0:NKT // 2, :],
                XT["k"].ap().rearrange("(kt p) t -> p kt t", p=P)[:, 0:NKT // 2, 0:512])
            w2b = {}
            w2b["k"] = persist.tile([P, NKT, F], BF16, name="w2b_k")
            nc.sync.dma_start(w2b["k"][:], W2["k"].ap().rearrange("(kt p) f -> p kt f", p=P))
            nc.sync.dma_start(
                xtb_k0[:, NKT // 2:, :],
                XT["k"].ap().rearrange("(kt p) t -> p kt t", p=P)[:, NKT // 2:, 0:512])
            for x in "vq":
                w2b[x] = persist.tile([P, NKT, F], BF16, name=f"w2b_{x}")
                nc.sync.dma_start(w2b[x][:], W2[x].ap().rearrange("(kt p) f -> p kt f", p=P))
            b2q_sb = persist.tile([F, 1], FP32, name="b2q_sb")
            nc.sync.dma_start(b2q_sb[:], b2q[:])
            b2k_sb = persist.tile([F, 1], FP32, name="b2k_sb")
            nc.sync.dma_start(b2k_sb[:], b2k[:])
            bvb_sb = persist.tile([P, HC * (HD + 1)], FP32, name="bvb_sb")
            nc.sync.dma_start(bvb_sb[:], bvb[:])
            bpT_sb = persist.tile([P, D // P], FP32, name="bpT_sb")
            nc.sync.dma_start(bpT_sb[:], bpT[:])
            wpTb = persist.tile([P, NKT, D], BF16, name="wpTb")

            def load_wpT(half):
                nc.sync.dma_start(
                    wpTb[:, :, half * 512:(half + 1) * 512],
                    WpT.ap().rearrange("(kt p) o -> p kt o", p=P)[:, :, half * 512:(half + 1) * 512],
                )
            # head selector: oh2[i, m] = (m // 64 == i); broadcasts [2,512] recips to [128,512]
            onehot2 = persist.tile([HC, P], FP32, name="onehot2")
            nc.sync.dma_start(onehot2[:], onehot_d[:])

            # persistent activations
            Qt = persist.tile([F, T], BF16, name="Qt")        # [2h*64, tok]
            Kt = persist.tile([F, T], BF16, name="Kt")
            Vx = persist.tile([P, T // P, HC * (HD + 1)], BF16, name="Vx")
            valT = persist.tile([F, T], BF16, name="valT")
            nc.vector.memset(Vx[:, :, HD:HD + 1], 1.0)        # ones columns
            nc.vector.memset(Vx[:, :, 2 * HD + 1:2 * HD + 2], 1.0)

            # ---------- phase A pieces (one 512-token chunk of one tensor) ----------
            def load_chunk(x, ch):
                t0 = ch * 512
                xtb = xtbp.tile([P, NKT, 512], BF16, name="xtb")
                nc.sync.dma_start(
                    xtb[:],
                    XT[x].ap().rearrange("(kt p) t -> p kt t", p=P)[:, :, t0:t0 + 512],
                )
                return xtb

            def proj_qk(x, ch, dest, sc, bias, xtb=None):
                t0 = ch * 512
                if xtb is None:
                    xtb = load_chunk(x, ch)
                ps = psA.tile([P, 1024], FP32, name="ps_st")
                for kt in range(NKT):
                    nc.tensor.matmul(ps[:, 0:512], lhsT=w2b[x][:, kt, :], rhs=xtb[:, kt, :],
                                     start=(kt == 0), stop=(kt == NKT - 1))
                nc.vector.tensor_scalar(dest[:, t0:t0 + 512], ps[:, 0:512], sc, bias[:, 0:1],
                                        op0=ALU.mult, op1=ALU.add)

            def proj_v(ch, xtb=None):
                if xtb is None:
                    xtb = load_chunk("v", ch)
                vps = psA.tile([P, 1024], FP32, name="ps_st")
                for sub in range(4):
                    for kt in range(NKT):
                        nc.tensor.matmul(vps[:, sub * F:(sub + 1) * F],
                                         lhsT=xtb[:, kt, sub * P:(sub + 1) * P],
                                         rhs=w2b["v"][:, kt, :],
                                         start=(kt == 0), stop=(kt == NKT - 1))
                for sub in range(4):
                    tt = ch * 4 + sub
                    for h in range(HC):
                        nc.vector.tensor_add(Vx[:, tt, h * 65:h * 65 + HD],
                                             vps[:, sub * F + h * HD:sub * F + (h + 1) * HD],
                                             bvb_sb[:, h * 65:h * 65 + HD])

            # ---------- Phase B ----------
            def phase_b(b, qc, extra_work=None):
                q0 = b * S + qc * 512
                vps = [psV.tile([P, 512], FP32, name="ps_val") for _ in range(HC)]
                est_prev = None
                for dk in range(NDK):
                    if extra_work is not None and dk in extra_work:
                        extra_work[dk]()
                    k0 = b * S + dk * 256
                    est = []
                    for h in range(HC):
                        fo = h * HD
                        stp = psA.tile([P, 1024], FP32, name="ps_st")
                        for half in range(2):
                            nc.tensor.matmul(stp[:, half * 512:(half + 1) * 512],
                                             lhsT=Kt[fo:fo + HD, k0 + half * P:k0 + (half + 1) * P],
                                             rhs=Qt[fo:fo + HD, q0:q0 + 512],
                                             start=True, stop=True)
                        e = estp.tile([P, 1024], BF16, name="est")
                        nc.scalar.activation(e[:], stp[:], AF.Exp)
                        est.append(e)
                    # val MMs for the previous double-tile (keeps PE fed while ACT runs)
                    if est_prev is not None:
                        emit_val(b, qc, dk - 1, est_prev, vps)
                    est_prev = est
                emit_val(b, qc, NDK - 1, est_prev, vps)
                # denominators -> den tile rows [2, 512]
                den_t = denp.tile([HC, 512], FP32, name="den")
                for h in range(HC):
                    dstage = small.tile([P, 512], FP32, name="dstage")
                    nc.vector.tensor_copy(dstage[HD:HD + 1, :], vps[h][HD:HD + 1, :])
                    nc.sync.dma_start(den_t[h:h + 1, :], dstage[HD:HD + 1, :])
                    # unnormalized val^T -> valT (bf16)
                    nc.vector.tensor_copy(valT[h * HD:(h + 1) * HD, q0:q0 + 512],
                                          vps[h][0:HD, :])
                return den_t

            def emit_val(b, qc, dk, est, vps):
                for h in range(HC):
                    for half in range(2):
                        kt = dk * 2 + half
                        nc.tensor.matmul(vps[h][0:HD + 1, :],
                                         lhsT=Vx[:, b * NST + kt, h * 65:(h + 1) * 65],
                                         rhs=est[h][:, half * 512:(half + 1) * 512],
                                         start=(kt == 0), stop=(kt == NST - 1))

            # norm: recip the denominators, broadcast via K=2 matmul, scale
            # valT in place, stage [M,F,64] (64-token strip per dst core)
            # into this chunk's slice of its consolidated a2a payload.
            def norm_chunk(b, qc, den_t):
                q0 = b * S + qc * 512
                i, s = (0, qc) if b == 0 else ((1, qc) if qc < 3 else (2, 0))
                rcp_t = rcpp.tile([HC, 512], FP32, name="rcp")
                nc.vector.reciprocal_approx_fast(rcp_t[:], den_t[:])
                rbp = psA.tile([P, 1024], FP32, name="ps_st")
                nc.tensor.matmul(rbp[:, 0:512], lhsT=onehot2[:], rhs=rcp_t[:], start=True, stop=True)
                nc.vector.tensor_mul(valT[:, q0:q0 + 512], rbp[:, 0:512], valT[:, q0:q0 + 512])
                nc.sync.dma_start(gin[i].ap().rearrange("j p s t -> p j s t")[:, :, s, :],
                                  valT[:, q0:q0 + 512].rearrange("p (j t) -> p j t", j=M))

            def a2a_send(i):
                nc.gpsimd.collective_compute(
                    "AllToAll", ALU.bypass, replica_groups=[list(range(M))],
                    ins=[gin[i][:]], outs=[gout[i][:]])

            # ---------- Phase D (tail only) ----------
            concatT = [persist.tile([P, NKT, NQC, 64], BF16, name=f"concatT{b}")
                       for b in range(B)]

            def load_concat(i):
                if i == 0:
                    dst = concatT[0][:]
                elif i == 1:
                    dst = concatT[1][:, :, 0:3, :]
                else:
                    dst = concatT[1][:, :, 3:4, :]
                nc.sync.dma_start(dst, gout[i].ap().rearrange("j p s t -> p j s t"))

            def phase_d_part(b, og, s0, s1):
                # sub-slots on a fixed 256-col stride: PSUM matmul outputs must
                # not cross a 2KB bank boundary
                n = (s1 - s0) * 64
                ops = psA.tile([P, 1024], FP32, name="ps_st")
                for sub in range(4):
                    oc = og * 4 + sub
                    for ft in range(NKT):
                        nc.tensor.matmul(ops[:, sub * 256:sub * 256 + n],
                                         lhsT=wpTb[:, ft, oc * P:(oc + 1) * P],
                                         rhs=concatT[b][:, ft, s0:s1, :],
                                         start=(ft == 0), stop=(ft == NKT - 1))
                ot = small.tile([P, 4, 256], FP32, name="ot")
                for sub in range(4):
                    oc = og * 4 + sub
                    nc.vector.tensor_scalar_add(ot[:, sub, 0:n],
                                                ops[:, sub * 256:sub * 256 + n],
                                                bpT_sb[:, oc:oc + 1])
                nc.gpsimd.dma_start(
                    outT.ap().rearrange("(og oc p) (bb s t) -> p og oc bb s t",
                                        p=P, oc=4, bb=B, t=64)[:, og, :, b, s0:s1, :],
                    ot[:, :, 0:n].rearrange("p oc (s t) -> p oc s t", t=64))

            # ---------- emission ----------
            # Attention starts after only k0+v0+q0; remaining projections are
            # interleaved into phase-B chunks as extra work, with their loads
            # pre-issued in consumption order so PE never stalls on a cast.
            # Chunk (b,qc)'s norm+a2a runs inside phase_b(b,qc+1) -- its den
            # DMA has ~5us to land. Nothing in phases A/B waits on any
            # collective; phase D (the only consumer) is at the tail.
            pre = {}
            den = {}

            def pk(x, ch):
                pre[f"{x}{ch}"] = load_chunk(x, ch)

            def pj(x, ch):
                if x == "v":
                    proj_v(ch, xtb=pre[f"v{ch}"])
                elif x == "k":
                    proj_qk("k", ch, Kt, 1.0, b2k_sb, xtb=pre[f"k{ch}"])
                else:
                    proj_qk("q", ch, Qt, SCALE, b2q_sb, xtb=pre[f"q{ch}"])

            def norm(b, qc):
                norm_chunk(b, qc, den[(b, qc)])

            proj_qk("k", 0, Kt, 1.0, b2k_sb, xtb=xtb_k0)
            proj_v(0)
            proj_qk("q", 0, Qt, SCALE, b2q_sb)
            pk("k", 1); pk("v", 1); pk("k", 2); pk("v", 2)
            den[(0, 0)] = phase_b(0, 0, extra_work={
                2: lambda: (pj("k", 1), pj("v", 1), pk("k", 3), pk("v", 3)),
                4: lambda: (pj("k", 2), pj("v", 2), pk("q", 1), pk("k", 4)),
                6: lambda: (pj("k", 3), pj("v", 3), pk("v", 4)),
            })
            pj("q", 1)
            den[(0, 1)] = phase_b(0, 1, extra_work={
                3: lambda: (pj("k", 4), pj("v", 4), pk("q", 2), pk("k", 5), pk("v", 5)),
                6: lambda: norm(0, 0),
            })
            load_wpT(0)
            pj("q", 2)
            den[(0, 2)] = phase_b(0, 2, extra_work={
                3: lambda: (pj("k", 5), pj("v", 5), pk("q", 3), pk("k", 6), pk("v", 6)),
                6: lambda: norm(0, 1),
            })
            load_wpT(1)
            pj("q", 3)
            den[(0, 3)] = phase_b(0, 3, extra_work={
                2: lambda: (pj("k", 6), pj("v", 6), pk("k", 7), pk("v", 7)),
                5: lambda: (pj("k", 7), pj("v", 7), pk("q", 4)),
                7: lambda: norm(0, 2),
            })
            pj("q", 4)
            pk("q", 5)
            den[(1, 0)] = phase_b(1, 0, extra_work={
                2: lambda: (norm(0, 3), a2a_send(0)),
                4: lambda: (pj("q", 5), pk("q", 6)),
            })
            den[(1, 1)] = phase_b(1, 1, extra_work={
                2: lambda: norm(1, 0),
                4: lambda: (pj("q", 6), pk("q", 7)),
            })
            den[(1, 2)] = phase_b(1, 2, extra_work={
                2: lambda: norm(1, 1),
                4: lambda: pj("q", 7),
                6: lambda: load_concat(0),
            })
            den[(1, 3)] = phase_b(1, 3, extra_work={
                2: lambda: (norm(1, 2), a2a_send(1)),
            })
            norm(1, 3)
            a2a_send(2)
            load_concat(1)
            phase_d_part(0, 0, 0, 4)
            phase_d_part(0, 1, 0, 4)
            phase_d_part(1, 0, 0, 3)
            phase_d_part(1, 1, 0, 3)
            load_concat(2)
            phase_d_part(1, 0, 3, 4)
            phase_d_part(1, 1, 3, 4)

    nc.compile()
    return nc


def _host_prep(inputs):
    f32 = np.float32
    bf16 = ml_dtypes.bfloat16
    QT = np.ascontiguousarray(inputs["Q_in"].reshape(T, D).T).astype(bf16)
    KT = np.ascontiguousarray(inputs["K_in"].reshape(T, D).T).astype(bf16)
    VT = np.ascontiguousarray(inputs["V_in"].reshape(T, D).T).astype(bf16)
    WpT = np.ascontiguousarray(inputs["Wp"].T).astype(bf16)
    bpT = np.ascontiguousarray(inputs["bp"].reshape(D // P, P).T).astype(f32, copy=False)
    oh2 = np.zeros((HC, P), f32)
    for h in range(HC):
        oh2[h, h * HD:(h + 1) * HD] = 1.0
    in_maps = []
    for c in range(M):
        sl = slice(c * HC, (c + 1) * HC)
        m = {
            "qT": QT, "kT": KT, "vT": VT, "WpT": WpT, "bpT": bpT, "onehot": oh2,
            "w2q": np.ascontiguousarray(inputs["Wq"][sl].transpose(1, 0, 2).reshape(D, F)).astype(bf16),
            "w2k": np.ascontiguousarray(inputs["Wk"][sl].transpose(1, 0, 2).reshape(D, F)).astype(bf16),
            "w2v": np.ascontiguousarray(inputs["Wv"][sl].transpose(1, 0, 2).reshape(D, F)).astype(bf16),
            "b2q": (inputs["bq"][sl].reshape(F, 1) * SCALE).astype(f32),
            "b2k": inputs["bk"][sl].reshape(F, 1).astype(f32),
        }
        bvb = np.zeros((P, HC * (HD + 1)), f32)
        for h in range(HC):
            bvb[:, h * 65:h * 65 + HD] = inputs["bv"][c * HC + h][None, :]
        m["bvb"] = bvb
        in_maps.append(m)
    return in_maps


_LAST = {"exec_time_ns": None}


def kernel(**inputs):
    inputs = {k: np.asarray(v) for k, v in inputs.items()}
    if "nc" not in _CACHE:
        _CACHE["nc"] = build()
    nc = _CACHE["nc"]
    in_maps = _host_prep(inputs)
    res = run_bass_kernel_spmd(nc, in_maps, core_ids=list(range(M)),
                               trace=_LAST.get("trace", False))
    _LAST["exec_time_ns"] = res.exec_time_ns
    _LAST["res"] = res
    out = np.zeros((T, D), np.float32)
    for c in range(M):
        oT = res.results[c]["outT"].reshape(D, B, NQC, 64)  # [D, b, qc, 64]
        for b in range(B):
            for qc in range(NQC):
                t0 = b * S + qc * 512 + c * 64
                out[t0:t0 + 64, :] = oT[:, b, qc, :].T
    return out.reshape(B, S, D)


# revision 24
# speedup vs baseline: 1.0169x; 1.0169x over previous
"""Trainium2 Bass kernel for nn_MultiHeadAttention (B=2,S=2048,D=1024,H=16,HD=64).

Sharding: tensor-parallel over heads (2 heads/core x 8 cores).
Per core:
  Phase A: load pre-transposed bf16 X^T (Q/K/V inputs), project to
           Qt/Kt [128(2h*64), 4096] (transposed, bf16) and V [4096, 2x(64+ones)] (bf16).
  Phase B: per (batch, q-chunk): scores^T[keys,q] = Kt_h-tiles @ Qt_h (PE, bf16),
           exp via ACT (scale folded into Qt; no max-subtraction -- scores are
           O(1) by construction), val^T[65,q] = V_ext^T @ exp_st (ones column ->
           row 64 = softmax denom), val MMs interleaved per double-key-tile.
  Norm:    per q-chunk (scheduled inside the NEXT chunk's phase B):
           reciprocal_approx_fast on [2,512] denoms, K=2 PE broadcast to
           [128,512], in-place DVE multiply into valT, then stage + per-chunk
           AllToAll [M,F,64] (each dst core gets its 64-token strip).
  Phase D: strictly at the tail (never blocks the attention PE stream on a
           collective): out^T[o, strip] = WpT-tiles @ concatT-strips + bp.
           Batch-0 strips + batch-1 strips 0-2 first; the last chunk's a2a
           flies under ~15us of phase-D work before its strip is consumed.
Host: pure layout prep (transposes/slices/bf16 cast) + output assembly.

Per-chunk collectives (8 x 128KB + warmup) give every a2a except the last
50-150us of slack against ncfw latency variance; nothing in the attention
stream waits on a collective.
"""
import sys
import numpy as np
import ml_dtypes

sys.path.insert(0, "/opt/trn_rl_repo")
sys.path.insert(0, "/opt/trn_rl_repo/concourse")

import concourse.bass as bass
import concourse.tile as tile
from concourse import bacc, mybir
from concourse.bass_utils import run_bass_kernel_spmd

FP32 = mybir.dt.float32
BF16 = mybir.dt.bfloat16
AF = mybir.ActivationFunctionType
ALU = mybir.AluOpType

B, S, D, H, HD = 2, 2048, 1024, 16, 64
M = 8                 # cores
HC = H // M           # heads per core
F = HC * HD           # 128 per-core proj features
T = B * S             # 4096 tokens
TS = T // M           # 512 tokens per core (final proj)
SR = S // M           # 256 seq rows per core per batch
SCALE = HD ** -0.5
P = 128
NKT = D // P          # 8 contraction tiles
NST = S // P          # 16 key tiles per batch
NDK = NST // 2        # 8 double key tiles
NQC = S // 512        # 4 q-chunks per batch
NCH = B * NQC         # 8 chunks total

_CACHE = {}


def build():
    nc = bacc.Bacc("TRN2", target_bir_lowering=False, debug=False, num_devices=M)

    XT = {x: nc.dram_tensor(f"{x}T", [D, T], BF16, kind="ExternalInput") for x in "qkv"}
    W2 = {x: nc.dram_tensor(f"w2{x}", [D, F], BF16, kind="ExternalInput") for x in "qkv"}
    b2q = nc.dram_tensor("b2q", [F, 1], FP32, kind="ExternalInput")
    b2k = nc.dram_tensor("b2k", [F, 1], FP32, kind="ExternalInput")
    bvb = nc.dram_tensor("bvb", [P, HC * (HD + 1)], FP32, kind="ExternalInput")
    WpT = nc.dram_tensor("WpT", [D, D], BF16, kind="ExternalInput")
    bpT = nc.dram_tensor("bpT", [P, D // P], FP32, kind="ExternalInput")
    onehot_d = nc.dram_tensor("onehot", [HC, P], FP32, kind="ExternalInput")
    outT = nc.dram_tensor("outT", [D, TS], FP32, kind="ExternalOutput")
    # 3 consolidated a2a payloads: batch0 (4 strips), batch1 strips 0-2, strip 3
    gin = [nc.dram_tensor(f"gin{i}", [M, F, ns, 64], BF16)
           for i, ns in enumerate((4, 3, 1))]
    gout = [nc.dram_tensor(f"gout{i}", [M, F, ns, 64], BF16)
            for i, ns in enumerate((4, 3, 1))]

    with tile.TileContext(nc) as tc:
        with (
            tc.tile_pool(name="persist", bufs=1) as persist,
            tc.tile_pool(name="xtb", bufs=6) as xtbp,
            tc.tile_pool(name="est", bufs=6) as estp,
            tc.tile_pool(name="small", bufs=4) as small,
            tc.tile_pool(name="den", bufs=6) as denp,
            tc.tile_pool(name="rcp", bufs=2) as rcpp,
            tc.tile_pool(name="ps_st", bufs=3, space="PSUM") as psA,   # [128,1024] = 2 banks
            tc.tile_pool(name="ps_val", bufs=2, space="PSUM") as psV,  # [128,512] = 1 bank
        ):
            # ---------- warmup collective first (absorbs ncfw cold-start) ----
            warm_in = nc.dram_tensor("cc_warm_in", [M, 1, 64], BF16)
            warm_out = nc.dram_tensor("cc_warm_out", [M, 1, 64], BF16)
            wtile = small.tile([1, M * 64], BF16, name="wtile")
            nc.vector.memset(wtile[:], 1.0)
            nc.gpsimd.dma_start(warm_in.ap().rearrange("j p r -> p (j r)"), wtile[:])
            nc.gpsimd.collective_compute(
                "AllToAll", ALU.bypass, replica_groups=[list(range(M))],
                ins=[warm_in[:]], outs=[warm_out[:]])

            # ---------- first-consumed loads first: k0/v0 interleaved halves
            xtb_k0 = xtbp.tile([P, NKT, 512], BF16, name="xtb")
            xtb_v0 = xtbp.tile([P, NKT, 512], BF16, name="xtb")
            xre = "(kt p) t -> p kt t"
            nc.sync.dma_start(xtb_k0[:, 0:4, :], XT["k"].ap().rearrange(xre, p=P)[:, 0:4, 0:512])
            w2b = {}
            for x in "kv":
                w2b[x] = persist.tile([P, NKT, F], BF16, name=f"w2b_{x}")
                nc.sync.dma_start(w2b[x][:], W2[x].ap().rearrange("(kt p) f -> p kt f", p=P))
            nc.sync.dma_start(xtb_k0[:, 4:, :], XT["k"].ap().rearrange(xre, p=P)[:, 4:, 0:512])
            # v0 halves split by token-sub so proj_v sub pairs start early
            nc.sync.dma_start(xtb_v0[:, :, 0:256], XT["v"].ap().rearrange(xre, p=P)[:, :, 0:256])
            nc.sync.dma_start(xtb_v0[:, :, 256:512], XT["v"].ap().rearrange(xre, p=P)[:, :, 256:512])
            w2b["q"] = persist.tile([P, NKT, F], BF16, name="w2b_q")
            nc.sync.dma_start(w2b["q"][:], W2["q"].ap().rearrange("(kt p) f -> p kt f", p=P))
            b2q_sb = persist.tile([F, 1], FP32, name="b2q_sb")
            nc.sync.dma_start(b2q_sb[:], b2q[:])
            b2k_sb = persist.tile([F, 1], FP32, name="b2k_sb")
            nc.sync.dma_start(b2k_sb[:], b2k[:])
            bvb_sb = persist.tile([P, HC * (HD + 1)], FP32, name="bvb_sb")
            nc.sync.dma_start(bvb_sb[:], bvb[:])
            bpT_sb = persist.tile([P, D // P], FP32, name="bpT_sb")
            nc.sync.dma_start(bpT_sb[:], bpT[:])
            wpTb = persist.tile([P, NKT, D], BF16, name="wpTb")

            def load_wpT(half):
                nc.sync.dma_start(
                    wpTb[:, :, half * 512:(half + 1) * 512],
                    WpT.ap().rearrange("(kt p) o -> p kt o", p=P)[:, :, half * 512:(half + 1) * 512],
                )
            # head selector: oh2[i, m] = (m // 64 == i); broadcasts [2,512] recips to [128,512]
            onehot2 = persist.tile([HC, P], FP32, name="onehot2")
            nc.sync.dma_start(onehot2[:], onehot_d[:])

            # persistent activations
            Qt = persist.tile([F, T], BF16, name="Qt")        # [2h*64, tok]
            Kt = persist.tile([F, T], BF16, name="Kt")
            Vx = persist.tile([P, T // P, HC * (HD + 1)], BF16, name="Vx")
            valT = persist.tile([F, T], BF16, name="valT")
            nc.vector.memset(Vx[:, :, HD:HD + 1], 1.0)        # ones columns
            nc.vector.memset(Vx[:, :, 2 * HD + 1:2 * HD + 2], 1.0)

            # ---------- phase A pieces (one 512-token chunk of one tensor) ----------
            def load_chunk(x, ch):
                t0 = ch * 512
                xtb = xtbp.tile([P, NKT, 512], BF16, name="xtb")
                nc.sync.dma_start(
                    xtb[:],
                    XT[x].ap().rearrange("(kt p) t -> p kt t", p=P)[:, :, t0:t0 + 512],
                )
                return xtb

            def proj_qk(x, ch, dest, sc, bias, xtb=None):
                t0 = ch * 512
                if xtb is None:
                    xtb = load_chunk(x, ch)
                ps = psA.tile([P, 1024], FP32, name="ps_st")
                for kt in range(NKT):
                    nc.tensor.matmul(ps[:, 0:512], lhsT=w2b[x][:, kt, :], rhs=xtb[:, kt, :],
                                     start=(kt == 0), stop=(kt == NKT - 1))
                nc.vector.tensor_scalar(dest[:, t0:t0 + 512], ps[:, 0:512], sc, bias[:, 0:1],
                                        op0=ALU.mult, op1=ALU.add)

            def proj_v_half(ch, half, xtb):
                vps = psA.tile([P, 1024], FP32, name="ps_st")
                for i in range(2):
                    sub = half * 2 + i
                    for kt in range(NKT):
                        nc.tensor.matmul(vps[:, i * F:(i + 1) * F],
                                         lhsT=xtb[:, kt, sub * P:(sub + 1) * P],
                                         rhs=w2b["v"][:, kt, :],
                                         start=(kt == 0), stop=(kt == NKT - 1))
                for i in range(2):
                    sub = half * 2 + i
                    tt = ch * 4 + sub
                    for h in range(HC):
                        nc.vector.tensor_add(Vx[:, tt, h * 65:h * 65 + HD],
                                             vps[:, i * F + h * HD:i * F + (h + 1) * HD],
                                             bvb_sb[:, h * 65:h * 65 + HD])

            def proj_v(ch, xtb=None):
                if xtb is None:
                    xtb = load_chunk("v", ch)
                proj_v_half(ch, 0, xtb)
                proj_v_half(ch, 1, xtb)

            # ---------- Phase B ----------
            def phase_b(b, qc, extra_work=None):
                q0 = b * S + qc * 512
                vps = [psV.tile([P, 512], FP32, name="ps_val") for _ in range(HC)]
                est_prev = None
                for dk in range(NDK):
                    if extra_work is not None and dk in extra_work:
                        extra_work[dk]()
                    k0 = b * S + dk * 256
                    est = []
                    for h in range(HC):
                        fo = h * HD
                        stp = psA.tile([P, 1024], FP32, name="ps_st")
                        for half in range(2):
                            nc.tensor.matmul(stp[:, half * 512:(half + 1) * 512],
                                             lhsT=Kt[fo:fo + HD, k0 + half * P:k0 + (half + 1) * P],
                                             rhs=Qt[fo:fo + HD, q0:q0 + 512],
                                             start=True, stop=True)
                        e = estp.tile([P, 1024], BF16, name="est")
                        nc.scalar.activation(e[:], stp[:], AF.Exp)
                        est.append(e)
                    # val MMs for the previous double-tile (keeps PE fed while ACT runs)
                    if est_prev is not None:
                        emit_val(b, qc, dk - 1, est_prev, vps)
                    est_prev = est
                emit_val(b, qc, NDK - 1, est_prev, vps)
                # denominators -> den tile rows [2, 512]
                den_t = denp.tile([HC, 512], FP32, name="den")
                for h in range(HC):
                    dstage = small.tile([P, 512], FP32, name="dstage")
                    nc.vector.tensor_copy(dstage[HD:HD + 1, :], vps[h][HD:HD + 1, :])
                    nc.sync.dma_start(den_t[h:h + 1, :], dstage[HD:HD + 1, :])
                    # unnormalized val^T -> valT (bf16)
                    nc.vector.tensor_copy(valT[h * HD:(h + 1) * HD, q0:q0 + 512],
                                          vps[h][0:HD, :])
                return den_t

            def emit_val(b, qc, dk, est, vps):
                for h in range(HC):
                    for half in range(2):
                        kt = dk * 2 + half
                        nc.tensor.matmul(vps[h][0:HD + 1, :],
                                         lhsT=Vx[:, b * NST + kt, h * 65:(h + 1) * 65],
                                         rhs=est[h][:, half * 512:(half + 1) * 512],
                                         start=(kt == 0), stop=(kt == NST - 1))

            # norm: recip the denominators, broadcast via K=2 matmul, scale
            # valT in place, stage [M,F,64] (64-token strip per dst core)
            # into this chunk's slice of its consolidated a2a payload.
            def norm_chunk(b, qc, den_t, pre_work=None):
                q0 = b * S + qc * 512
                i, s = (0, qc) if b == 0 else ((1, qc) if qc < 3 else (2, 0))
                rcp_t = rcpp.tile([HC, 512], FP32, name="rcp")
                nc.vector.reciprocal_approx_fast(rcp_t[:], den_t[:])
                if pre_work is not None:
                    pre_work()  # PE filler emitted ahead of rbp in the queue
                rbp = psA.tile([P, 1024], FP32, name="ps_st")
                nc.tensor.matmul(rbp[:, 0:512], lhsT=onehot2[:], rhs=rcp_t[:], start=True, stop=True)
                nc.vector.tensor_mul(valT[:, q0:q0 + 512], rbp[:, 0:512], valT[:, q0:q0 + 512])
                nc.sync.dma_start(gin[i].ap().rearrange("j p s t -> p j s t")[:, :, s, :],
                                  valT[:, q0:q0 + 512].rearrange("p (j t) -> p j t", j=M))

            def a2a_send(i):
                nc.gpsimd.collective_compute(
                    "AllToAll", ALU.bypass, replica_groups=[list(range(M))],
                    ins=[gin[i][:]], outs=[gout[i][:]])

            # ---------- Phase D (tail only) ----------
            concatT = [persist.tile([P, NKT, NQC, 64], BF16, name=f"concatT{b}")
                       for b in range(B)]

            def load_concat(i):
                if i == 0:
                    dst = concatT[0][:]
                elif i == 1:
                    dst = concatT[1][:, :, 0:3, :]
                else:
                    dst = concatT[1][:, :, 3:4, :]
                nc.sync.dma_start(dst, gout[i].ap().rearrange("j p s t -> p j s t"))

            def phase_d_piece(b, og, sp, s0=0, s1=NQC):
                # two output-column blocks per piece; sub-slots on a fixed
                # 256-col stride (PSUM matmul outputs must not cross a 2KB
                # bank boundary)
                n = (s1 - s0) * 64
                ops = psA.tile([P, 1024], FP32, name="ps_st")
                for i in range(2):
                    oc = og * 4 + sp * 2 + i
                    for ft in range(NKT):
                        nc.tensor.matmul(ops[:, i * 256:i * 256 + n],
                                         lhsT=wpTb[:, ft, oc * P:(oc + 1) * P],
                                         rhs=concatT[b][:, ft, s0:s1, :],
                                         start=(ft == 0), stop=(ft == NKT - 1))
                ot = small.tile([P, 2, 256], FP32, name="ot")
                for i in range(2):
                    oc = og * 4 + sp * 2 + i
                    nc.vector.tensor_scalar_add(ot[:, i, 0:n],
                                                ops[:, i * 256:i * 256 + n],
                                                bpT_sb[:, oc:oc + 1])
                nc.gpsimd.dma_start(
                    outT.ap().rearrange("(og oc p) (bb s t) -> p og oc bb s t",
                                        p=P, oc=4, bb=B, t=64)[:, og, sp * 2:sp * 2 + 2, b, s0:s1, :],
                    ot[:, :, 0:n].rearrange("p oc (s t) -> p oc s t", t=64))

            def phase_d_part(b, og, s0, s1):
                phase_d_piece(b, og, 0, s0, s1)
                phase_d_piece(b, og, 1, s0, s1)

            # ---------- emission ----------
            # Attention starts after only k0+v0+q0; remaining projections are
            # interleaved into phase-B chunks as extra work, with their loads
            # pre-issued in consumption order so PE never stalls on a cast.
            # Chunk (b,qc)'s norm+a2a runs inside phase_b(b,qc+1) -- its den
            # DMA has ~5us to land. Nothing in phases A/B waits on any
            # collective; phase D (the only consumer) is at the tail.
            pre = {}
            den = {}

            def pk(x, ch):
                pre[f"{x}{ch}"] = load_chunk(x, ch)

            def pj(x, ch):
                if x == "v":
                    proj_v(ch, xtb=pre[f"v{ch}"])
                elif x == "k":
                    proj_qk("k", ch, Kt, 1.0, b2k_sb, xtb=pre[f"k{ch}"])
                else:
                    proj_qk("q", ch, Qt, SCALE, b2q_sb, xtb=pre[f"q{ch}"])

            def norm(b, qc):
                norm_chunk(b, qc, den[(b, qc)])

            def pvh(ch, half):
                proj_v_half(ch, half, pre[f"v{ch}"])

            proj_qk("k", 0, Kt, 1.0, b2k_sb, xtb=xtb_k0)
            proj_v(0, xtb=xtb_v0)
            proj_qk("q", 0, Qt, SCALE, b2q_sb)
            pk("k", 1); pk("v", 1); pk("k", 2); pk("v", 2)
            den[(0, 0)] = phase_b(0, 0, extra_work={
                1: lambda: pj("k", 1),
                2: lambda: (pvh(1, 0), pk("k", 3)),
                3: lambda: (pvh(1, 1), pk("v", 3)),
                4: lambda: (pj("k", 2), pk("q", 1)),
                5: lambda: (pvh(2, 0), pk("k", 4)),
                6: lambda: (pj("k", 3), pvh(2, 1), pk("v", 4)),
                7: lambda: (pvh(3, 0), pvh(3, 1)),
            })
            pj("q", 1)
            den[(0, 1)] = phase_b(0, 1, extra_work={
                1: lambda: (pj("k", 4), pk("k", 5)),
                3: lambda: (pvh(4, 0), pk("v", 5)),
                5: lambda: (pvh(4, 1), pk("q", 2)),
                6: lambda: norm(0, 0),
                7: lambda: pj("k", 5),
            })
            load_wpT(0)
            pj("q", 2)
            den[(0, 2)] = phase_b(0, 2, extra_work={
                1: lambda: (pvh(5, 0), pk("k", 6)),
                3: lambda: (pvh(5, 1), pk("v", 6)),
                5: lambda: (pj("k", 6), pk("q", 3)),
                6: lambda: norm(0, 1),
                7: lambda: pvh(6, 0),
            })
            load_wpT(1)
            pj("q", 3)
            den[(0, 3)] = phase_b(0, 3, extra_work={
                1: lambda: (pvh(6, 1), pk("k", 7)),
                3: lambda: (pj("k", 7), pk("v", 7)),
                5: lambda: (pvh(7, 0), pk("q", 4)),
                6: lambda: norm(0, 2),
                7: lambda: pvh(7, 1),
            })
            pj("q", 4)
            pk("q", 5)
            den[(1, 0)] = phase_b(1, 0, extra_work={
                2: lambda: (norm(0, 3), a2a_send(0)),
                4: lambda: (pj("q", 5), pk("q", 6)),
            })
            den[(1, 1)] = phase_b(1, 1, extra_work={
                2: lambda: norm(1, 0),
                4: lambda: (pj("q", 6), pk("q", 7)),
            })
            den[(1, 2)] = phase_b(1, 2, extra_work={
                2: lambda: norm(1, 1),
                3: lambda: load_concat(0),
                4: lambda: pj("q", 7),
                6: lambda: phase_d_piece(0, 0, 0),
            })
            den[(1, 3)] = phase_b(1, 3, extra_work={
                2: lambda: (norm(1, 2), a2a_send(1)),
                4: lambda: phase_d_piece(0, 0, 1),
                6: lambda: phase_d_piece(0, 1, 0),
            })
            norm_chunk(1, 3, den[(1, 3)],
                       pre_work=lambda: phase_d_piece(0, 1, 1))
            a2a_send(2)
            load_concat(1)
            phase_d_part(1, 0, 0, 3)
            phase_d_part(1, 1, 0, 3)
            load_concat(2)
            phase_d_part(1, 0, 3, 4)
            phase_d_part(1, 1, 3, 4)

    nc.compile()
    return nc


def _host_prep(inputs):
    f32 = np.float32
    bf16 = ml_dtypes.bfloat16
    QT = np.ascontiguousarray(inputs["Q_in"].reshape(T, D).T).astype(bf16)
    KT = np.ascontiguousarray(inputs["K_in"].reshape(T, D).T).astype(bf16)
    VT = np.ascontiguousarray(inputs["V_in"].reshape(T, D).T).astype(bf16)
    WpT = np.ascontiguousarray(inputs["Wp"].T).astype(bf16)
    bpT = np.ascontiguousarray(inputs["bp"].reshape(D // P, P).T).astype(f32, copy=False)
    oh2 = np.zeros((HC, P), f32)
    for h in range(HC):
        oh2[h, h * HD:(h + 1) * HD] = 1.0
    in_maps = []
    for c in range(M):
        sl = slice(c * HC, (c + 1) * HC)
        m = {
            "qT": QT, "kT": KT, "vT": VT, "WpT": WpT, "bpT": bpT, "onehot": oh2,
            "w2q": np.ascontiguousarray(inputs["Wq"][sl].transpose(1, 0, 2).reshape(D, F)).astype(bf16),
            "w2k": np.ascontiguousarray(inputs["Wk"][sl].transpose(1, 0, 2).reshape(D, F)).astype(bf16),
            "w2v": np.ascontiguousarray(inputs["Wv"][sl].transpose(1, 0, 2).reshape(D, F)).astype(bf16),
            "b2q": (inputs["bq"][sl].reshape(F, 1) * SCALE).astype(f32),
            "b2k": inputs["bk"][sl].reshape(F, 1).astype(f32),
        }
        bvb = np.zeros((P, HC * (HD + 1)), f32)
        for h in range(HC):
            bvb[:, h * 65:h * 65 + HD] = inputs["bv"][c * HC + h][None, :]
        m["bvb"] = bvb
        in_maps.append(m)
    return in_maps


_LAST = {"exec_time_ns": None}


def kernel(**inputs):
    inputs = {k: np.asarray(v) for k, v in inputs.items()}
    if "nc" not in _CACHE:
        _CACHE["nc"] = build()
    nc = _CACHE["nc"]
    in_maps = _host_prep(inputs)
    res = run_bass_kernel_spmd(nc, in_maps, core_ids=list(range(M)),
                               trace=_LAST.get("trace", False))
    _LAST["exec_time_ns"] = res.exec_time_ns
    _LAST["res"] = res
    out = np.zeros((T, D), np.float32)
    for c in range(M):
        oT = res.results[c]["outT"].reshape(D, B, NQC, 64)  # [D, b, qc, 64]
        for b in range(B):
            for qc in range(NQC):
                t0 = b * S + qc * 512 + c * 64
                out[t0:t0 + 64, :] = oT[:, b, qc, :].T
    return out.reshape(B, S, D)
